# revision 2
# baseline (speedup 1.0000x reference)
"""Pointer-generator head on 8 Trainium2 NeuronCores (Bass/Tile).

Computation (per batch row b):
    p_gen = sigmoid(context @ w_c + state @ w_s + emb @ w_y + b)
    out   = p_gen * vocab_dist
    out[b, src_ids[b, t]] += (1 - p_gen) * attn_dist[b, t]   (masked, clamped)

Sharding: batch dim (512) split across 8 cores, 64 rows each; every core keeps
its rows' full V=32000 columns so the scatter-add stays core-local; the small
parameter vectors are replicated.

Per-core device kernel (interleaved layout: partition p = 2b+h holds row b's
half-row h = [h*16000, (h+1)*16000) contiguously):

  * p_gen: the host supplies [context|state|emb]^T with each column duplicated
    (column 2b+h = row b), so 20 accumulating PE matmuls produce the [128,1]
    per-partition gating scalars directly; sigmoid on the scalar engine.
  * dense: 4 stream chunks of [128, 4000] f32; scale by p_gen on the scalar
    engine (activation Copy with per-partition scale).
  * scatter: the host buckets the T=400 items per row by 1000-column target
    bucket (16 per partition) and window-packs duplicate-id groups into
    width-W windows; the device computes per-item duplicate-group sums with a
    windowed equality mask (3 vector ops).  Each bucket's group sums are then
    written into an int16 sparse tile with GPSIMD local_scatter, using the
    f32 group sums reinterpreted as (lo16, hi16) pairs — an exact f32 scatter
    with no extra data prep.  Non-first group members, masked items, and
    padding carry index -1 and are dropped.  One f32 add folds each chunk's
    sparse tile into the scaled dense chunk before the store.

Host-side work is limited to index metadata (permutations, bucket indices)
and pure data relayout (sharding, transposition, duplication of columns).
"""

import numpy as np

import concourse.bacc as bacc
import concourse.mybir as mybir
import concourse.tile as tile
from concourse import bass_utils

# ---- problem shape (hardcoded per spec) ----
B = 512
T = 400
V = 32000
ENC, HID, EMB = 1024, 1024, 512
NCORES = 8

P = 128
BSH = B // NCORES       # 64 rows per core
HV = V // 2             # half-row width per partition
W = 10                  # equality window width
D = ENC + HID + EMB     # 2560
NB = 16                 # scatter buckets per partition
BW = HV // NB           # 1000 f32 columns per bucket
NSTREAM = 4
SW = HV // NSTREAM      # 4000 f32 per partition per stream chunk

F32 = mybir.dt.float32
I16 = mybir.dt.int16


# --------------------------------------------------------------------------
# host-side index prep (pure metadata / relayout)
# --------------------------------------------------------------------------

def _bucketize(src_ids: np.ndarray, vocab_size: int):
    """Bucket one shard's unmasked items by (partition, bucket).

    Returns buckets[p][c] = list of (bucket-local target, [t indices]) groups;
    each group shares one raw id.  Masked items (id >= min(vocab_size, V))
    contribute nothing and are dropped.
    """
    id_lim = min(int(vocab_size), V)
    buckets = [[[] for _ in range(NB)] for _ in range(P)]
    for b in range(BSH):
        order: dict[int, list[int]] = {}
        for t, i in enumerate(src_ids[b].tolist()):
            if i < id_lim:
                order.setdefault(i, []).append(t)
        for i, g in order.items():
            h, off = divmod(i, HV)
            c = off // BW
            buckets[2 * b + h][c].append((off - c * BW, g))
    return buckets


def _pack_bucket(groups):
    """First-fit-decreasing of duplicate-id groups into windows of W.

    Returns (placements, nwin); placements is a list of
    (slot_offset_within_segment, loc, [t indices])."""
    fills: list[int] = []
    placements = []
    for loc, ts in sorted(groups, key=lambda g: -len(g[1])):
        if len(ts) > W:
            raise ValueError(f"duplicate group of {len(ts)} exceeds window {W}")
        for wdx, f in enumerate(fills):
            if f + len(ts) <= W:
                break
        else:
            wdx = len(fills)
            fills.append(0)
        placements.append((wdx * W + fills[wdx], loc, ts))
        fills[wdx] += len(ts)
    return placements, len(fills)


def _prep_shard(attn: np.ndarray, src_ids: np.ndarray, vocab_size: int, S: int):
    """-> (attn_p, ids_f, lsidx): [P, NB*S] f32, [P, NB*S] f32, [P, NB*2S] i16.

    Slot layout per partition: NB segments of S slots (S//W windows each);
    every duplicate-id group occupies consecutive slots inside one window.
    lsidx holds int16 index PAIRS per slot: the first member of group at slot
    j targeting bucket-local f32 column loc gets (2*loc, 2*loc+1) at positions
    (2j, 2j+1) — the scatter writes the f32 group sum as two int16 halves.
    Everything else is -1 (dropped)."""
    TS = NB * S
    attn_p = np.zeros((P, TS), np.float32)
    ids_f = np.full((P, TS), -1.0, np.float32)
    lsidx = np.full((P, 2 * TS), -1, np.int16)
    buckets = _bucketize(src_ids, vocab_size)
    for p in range(P):
        row = p // 2
        for c in range(NB):
            placements, nwin = _pack_bucket(buckets[p][c])
            assert nwin * W <= S
            for slot, loc, ts in placements:
                j = c * S + slot
                gid = float(src_ids[row, ts[0]])
                for k, t in enumerate(ts):
                    attn_p[p, j + k] = attn[row, t]
                    ids_f[p, j + k] = gid
                lsidx[p, 2 * j] = 2 * loc
                lsidx[p, 2 * j + 1] = 2 * loc + 1
    return attn_p, ids_f, lsidx


def _slot_requirement(src_ids_full: np.ndarray, vocab_size: int):
    """Global S: max windows used by any (core, partition, bucket)."""
    mx = 1
    for c in range(NCORES):
        buckets = _bucketize(src_ids_full[c * BSH : (c + 1) * BSH], vocab_size)
        for p in range(P):
            for ch in range(NB):
                _, nwin = _pack_bucket(buckets[p][ch])
                mx = max(mx, nwin)
    return mx * W


# --------------------------------------------------------------------------
# device kernel (per core; SPMD across 8 cores)
# --------------------------------------------------------------------------

def _build_kernel(tc: tile.TileContext, out, ins, b_const: float, S: int,
                  variant: str = "full"):
    nc = tc.nc
    do_sparse = variant in ("full", "noscat", "lsonly")
    do_pg = variant in ("full", "pg", "noscat")
    do_dense = variant != "lsonly"
    use_ls = variant != "noscat"
    vd, xdup, wall, attn_p, ids_f, lsidx = ins
    TS = NB * S
    NWT = TS // W
    NK = D // P  # K-chunks for the p_gen matmul

    with tc.tile_pool(name="small", bufs=1) as sp, \
         tc.tile_pool(name="psum", bufs=1, space="PSUM") as pp, \
         tc.tile_pool(name="stream", bufs=3) as pool, \
         tc.tile_pool(name="sparse", bufs=2) as spp:
        if do_pg:
            # ---- raw windowed duplicate-group sums (independent of p_gen) ----
            at = sp.tile([P, TS], F32)
            nc.sync.dma_start(at[:], attn_p[:, :])
            idt = sp.tile([P, TS], F32)
            nc.sync.dma_start(idt[:], ids_f[:, :])
            lsi = sp.tile([P, 2 * TS], I16)
            nc.sync.dma_start(lsi[:], lsidx[:, :])

            idw = idt[:].rearrange("p (w i) -> p w i", i=W)
            id_i = idw[:, :, :, None].to_broadcast([P, NWT, W, W])
            id_j = idw[:, :, None, :].to_broadcast([P, NWT, W, W])
            eq = sp.tile([P, NWT * W * W], F32)
            eqv = eq[:].rearrange("p (w i j) -> p w i j", i=W, j=W)
            nc.vector.tensor_tensor(eqv, id_i, id_j, op=mybir.AluOpType.is_equal)
            at_j = (
                at[:]
                .rearrange("p (w i) -> p w i", i=W)[:, :, None, :]
                .to_broadcast([P, NWT, W, W])
            )
            nc.vector.tensor_mul(eqv, eqv, at_j)
            gs = sp.tile([P, TS], F32)
            nc.vector.reduce_sum(
                gs[:].rearrange("p (w i) -> p w i", i=W),
                eqv,
                axis=mybir.AxisListType.X,
            )
            gs16 = gs[:].bitcast(I16)  # [P, 2*TS], (lo16, hi16) per f32 slot

            # ---- p_gen = sigmoid(xdup @ wall + b) via PE, column-duplicated:
            # xdup[:, 2b+h] = x[b] so dots land directly in interleaved [128,1]
            xT = sp.tile([P, NK * P], F32)
            nc.sync.dma_start(xT[:], xdup.rearrange("(k p) m -> p k m", p=P))
            wtile = sp.tile([P, NK], F32)
            nc.sync.dma_start(wtile[:], wall.rearrange("(k p) -> p k", p=P))
            dots_ps = pp.tile([P, 1], F32, space="PSUM")
            for k in range(NK):
                nc.tensor.matmul(
                    dots_ps[:],
                    lhsT=xT[:, k * P : (k + 1) * P],
                    rhs=wtile[:, k : k + 1],
                    start=(k == 0),
                    stop=(k == NK - 1),
                )
            pgd = sp.tile([P, 1], F32)
            nc.scalar.activation(
                pgd[:], dots_ps[:], mybir.ActivationFunctionType.Sigmoid,
                bias=b_const,
            )
            omd = sp.tile([P, 1], F32)  # 1 - p_gen
            nc.vector.tensor_scalar(
                omd[:], pgd[:], -1.0, 1.0,
                mybir.AluOpType.mult, mybir.AluOpType.add,
            )
        else:
            pgd = sp.tile([P, 1], F32)
            nc.vector.memset(pgd[:], 0.5)
            omd = sp.tile([P, 1], F32)
            nc.vector.memset(omd[:], 0.5)
            gs = sp.tile([P, TS], F32)
            nc.vector.memset(gs[:], 0.0)
            gs16 = gs[:].bitcast(I16)
            lsi = sp.tile([P, 2 * TS], I16)
            nc.sync.dma_start(lsi[:], lsidx[:, :])

        # ---- stream: out = p_gen * vocab_dist + sparse ----
        vdv = vd.rearrange("(p v) -> p v", p=P)
        outv = out.rearrange("(p v) -> p v", p=P)
        NSUB = SW // BW  # buckets per stream chunk
        for c in range(NSTREAM):
            tl = pool.tile([P, SW], F32, tag="stream")
            if do_dense:
                nc.sync.dma_start(tl[:], vdv[:, c * SW : (c + 1) * SW])
                nc.scalar.activation(
                    tl[:], tl[:], mybir.ActivationFunctionType.Copy, scale=pgd[:]
                )
            else:
                nc.vector.memset(tl[:], 0.0)
            if do_sparse:
                spt = spp.tile([P, 2 * SW], I16, tag="spt")
                if use_ls:
                    for sub in range(NSUB):
                        cc = c * NSUB + sub
                        nc.gpsimd.local_scatter(
                            out_ap=spt[:, sub * 2 * BW : (sub + 1) * 2 * BW],
                            data_ap=gs16[:, cc * 2 * S : (cc + 1) * 2 * S],
                            idxs_ap=lsi[:, cc * 2 * S : (cc + 1) * 2 * S],
                            channels=P, num_elems=2 * BW, num_idxs=2 * S,
                        )
                else:
                    nc.vector.memset(spt[:], 0)
                # tl = (sparse * (1 - p_gen)) + tl, fused on DVE
                nc.vector.scalar_tensor_tensor(
                    tl[:], spt[:].bitcast(F32), omd[:], tl[:],
                    op0=mybir.AluOpType.mult, op1=mybir.AluOpType.add,
                )
            if do_dense:
                nc.sync.dma_start(outv[:, c * SW : (c + 1) * SW], tl[:])
            else:
                nc.sync.dma_start(outv[:, c * 16 : (c + 1) * 16], tl[:, :16])


# --------------------------------------------------------------------------
# entry point
# --------------------------------------------------------------------------

last_results = None  # BassKernelResults of the most recent run (for benchmarks)


def build_program(b_const: float, S: int, repeat: int = 1, variant: str = "full"):
    nc = bacc.Bacc("TRN2", target_bir_lowering=False, debug=False,
                   num_devices=NCORES)
    vd_t = nc.dram_tensor("vd", [BSH * V], F32, kind="ExternalInput")
    xdup_t = nc.dram_tensor("xdup", [D, P], F32, kind="ExternalInput")
    wall_t = nc.dram_tensor("wall", [D], F32, kind="ExternalInput")
    attn_t = nc.dram_tensor("attn_p", [P, NB * S], F32, kind="ExternalInput")
    ids_t = nc.dram_tensor("ids_f", [P, NB * S], F32, kind="ExternalInput")
    lsi_t = nc.dram_tensor("lsidx", [P, NB * 2 * S], I16, kind="ExternalInput")
    out_t = nc.dram_tensor("out", [BSH * V], F32, kind="ExternalOutput")

    with tile.TileContext(nc) as tc:
        for _ in range(repeat):
            _build_kernel(
                tc,
                out_t.ap(),
                (vd_t.ap(), xdup_t.ap(), wall_t.ap(), attn_t.ap(), ids_t.ap(),
                 lsi_t.ap()),
                b_const,
                S,
                variant,
            )
    nc.compile()
    return nc


def prepare_in_maps(vocab_dist, attn_dist, xcat_full, wall_np, src_ids, vs, S):
    in_maps = []
    for c in range(NCORES):
        sl = slice(c * BSH, (c + 1) * BSH)
        attn_p, ids_f, lsidx = _prep_shard(attn_dist[sl], src_ids[sl], vs, S)
        xdup = np.ascontiguousarray(
            np.repeat(xcat_full[sl].T, 2, axis=1)  # column 2b+h = row b
        )
        in_maps.append(
            {
                "vd": np.ascontiguousarray(vocab_dist[sl]).reshape(-1),
                "xdup": xdup,
                "wall": wall_np,
                "attn_p": attn_p,
                "ids_f": ids_f,
                "lsidx": lsidx,
            }
        )
    return in_maps


def kernel(vocab_dist, attn_dist, context, state, emb, src_ids, vocab_size,
           w_c, w_s, w_y, b, **kwargs):
    vocab_dist = np.ascontiguousarray(np.asarray(vocab_dist, dtype=np.float32))
    attn_dist = np.asarray(attn_dist, dtype=np.float32)
    xcat_full = np.ascontiguousarray(
        np.concatenate(
            [np.asarray(context), np.asarray(state), np.asarray(emb)], axis=1
        ).astype(np.float32)
    )
    src_ids = np.asarray(src_ids)
    vs = int(np.asarray(vocab_size))
    wall_np = np.ascontiguousarray(
        np.concatenate(
            [np.asarray(w_c), np.asarray(w_s), np.asarray(w_y)]
        ).astype(np.float32)
    )
    b_const = float(np.asarray(b).reshape(-1)[0])

    assert vocab_dist.shape == (B, V) and attn_dist.shape == (B, T)
    assert xcat_full.shape == (B, D) and src_ids.shape == (B, T)

    S = _slot_requirement(src_ids, vs)
    nc = build_program(b_const, S)
    in_maps = prepare_in_maps(
        vocab_dist, attn_dist, xcat_full, wall_np, src_ids, vs, S
    )

    import os as _os
    _trace = _os.environ.get("PG_KERNEL_TRACE", "0") == "1"
    res = bass_utils.run_bass_kernel_spmd(
        nc, in_maps, core_ids=list(range(NCORES)), trace=_trace
    )
    global last_results
    last_results = res

    out = np.empty((B, V), np.float32)
    for c in range(NCORES):
        out[c * BSH : (c + 1) * BSH] = res.results[c]["out"].reshape(BSH, V)
    return out



# revision 3
# speedup vs baseline: 1.3391x; 1.3391x over previous
"""Pointer-generator head on 8 Trainium2 NeuronCores (Bass/Tile).

Computation (per batch row b):
    p_gen = sigmoid(context @ w_c + state @ w_s + emb @ w_y + b)
    out   = p_gen * vocab_dist
    out[b, src_ids[b, t]] += (1 - p_gen) * attn_dist[b, t]   (masked, clamped)

Sharding: batch dim (512) split across 8 cores, 64 rows each; every core keeps
its rows' full V=32000 columns so the scatter-add stays core-local; the small
parameter vectors are replicated.

Per-core device kernel (interleaved layout: partition p = 2b+h holds row b's
half-row h = [h*16000, (h+1)*16000) contiguously):

  * p_gen: 20 accumulating f32 PE matmuls produce per-row dots [64,1]; a 0/1
    duplication matmul expands them to the interleaved [128,1] layout; sigmoid
    on the scalar engine.
  * scatter prep: the host buckets the T=400 items per row by 2000-column
    target bucket (8 per partition) and window-packs duplicate-id groups into
    width-W windows, tagging each slot with its group's window-local first
    slot (a small integer, bf16-exact).  The device computes per-group sums
    with a windowed equality mask in bf16 (is_equal + mul, f32 reduce), then
    scales them by (1 - p_gen) into bf16 group sums.
  * stream: 8 chunks of [128, 2000] f32.  Per chunk, one GPSIMD local_scatter
    writes the chunk's bf16 group sums into a bf16 sparse tile (non-first
    group members, masked items and padding carry index -1 and are dropped);
    one DVE scalar_tensor_tensor computes p_gen * dense + sparse; the result
    streams back out.  Input DMAs are issued from the SP queue, output DMAs
    from the scalar-engine queue so neither blocks the other.

Host-side work is limited to index metadata (permutations, bucket indices,
0/1 selection matrices) and pure data relayout (sharding, transposition,
dtype casts).
"""

import os

import ml_dtypes
import numpy as np

import concourse.bacc as bacc
import concourse.mybir as mybir
import concourse.tile as tile
from concourse import bass_utils

# ---- problem shape (hardcoded per spec) ----
B = 512
T = 400
V = 32000
ENC, HID, EMB = 1024, 1024, 512
NCORES = 8

P = 128
BSH = B // NCORES       # 64 rows per core
HV = V // 2             # half-row width per partition
W = 10                  # equality window width
D = ENC + HID + EMB     # 2560
NK = D // P             # K-chunks for the p_gen matmul
NB = 8                  # scatter buckets per partition
BW = HV // NB           # 2000 f32 columns per bucket
NSTREAM = 8
SW = HV // NSTREAM      # 2000 f32 per partition per stream chunk (== BW)

F32 = mybir.dt.float32
BF16 = mybir.dt.bfloat16
I16 = mybir.dt.int16

NPBF16 = ml_dtypes.bfloat16


# --------------------------------------------------------------------------
# host-side index prep (pure metadata / relayout)
# --------------------------------------------------------------------------

def _bucketize(src_ids: np.ndarray, vocab_size: int):
    """Bucket one shard's unmasked items by (partition, bucket).

    Returns buckets[p][c] = list of (bucket-local target, [t indices]) groups;
    each group shares one raw id.  Masked items (id >= min(vocab_size, V))
    contribute nothing and are dropped.
    """
    id_lim = min(int(vocab_size), V)
    buckets = [[[] for _ in range(NB)] for _ in range(P)]
    for b in range(BSH):
        order: dict[int, list[int]] = {}
        for t, i in enumerate(src_ids[b].tolist()):
            if i < id_lim:
                order.setdefault(i, []).append(t)
        for i, g in order.items():
            h, off = divmod(i, HV)
            c = off // BW
            buckets[2 * b + h][c].append((off - c * BW, g))
    return buckets


def _pack_bucket(groups):
    """First-fit-decreasing of duplicate-id groups into windows of W.

    Returns (placements, nwin); placements is a list of
    (slot_offset_within_segment, loc, [t indices])."""
    fills: list[int] = []
    placements = []
    for loc, ts in sorted(groups, key=lambda g: -len(g[1])):
        if len(ts) > W:
            raise ValueError(f"duplicate group of {len(ts)} exceeds window {W}")
        for wdx, f in enumerate(fills):
            if f + len(ts) <= W:
                break
        else:
            wdx = len(fills)
            fills.append(0)
        placements.append((wdx * W + fills[wdx], loc, ts))
        fills[wdx] += len(ts)
    return placements, len(fills)


def _prep_shard(attn: np.ndarray, src_ids: np.ndarray, vocab_size: int, S: int):
    """-> (attn_p, tags, lsidx): [P, NB*S] bf16, [P, NB*S] bf16, [P, NB*S] i16.

    Slot layout per partition: NB segments of S slots (S//W windows each);
    every duplicate-id group occupies consecutive slots inside one window.
    tags[p, j] is the window-local slot index of slot j's group's first
    member (pad slots carry -1; their attn is 0 so they sum to 0 and their
    index is -1 so the scatter drops them).  lsidx[p, j] holds the group's
    bucket-local f32 target column for first members, else -1."""
    TS = NB * S
    attn_f = np.zeros((P, TS), np.float32)
    tags_f = np.full((P, TS), -1.0, np.float32)
    lsidx = np.full((P, TS), -1, np.int16)
    buckets = _bucketize(src_ids, vocab_size)
    for p in range(P):
        row = p // 2
        for c in range(NB):
            placements, nwin = _pack_bucket(buckets[p][c])
            assert nwin * W <= S
            for slot, loc, ts in placements:
                j = c * S + slot
                tag = float(slot % W)
                for k, t in enumerate(ts):
                    attn_f[p, j + k] = attn[row, t]
                    tags_f[p, j + k] = tag
                lsidx[p, j] = loc
    return attn_f.astype(NPBF16), tags_f.astype(NPBF16), lsidx


def _slot_requirement(src_ids_full: np.ndarray, vocab_size: int):
    """Global S: max windows used by any (core, partition, bucket)."""
    mx = 1
    for c in range(NCORES):
        buckets = _bucketize(src_ids_full[c * BSH : (c + 1) * BSH], vocab_size)
        for p in range(P):
            for ch in range(NB):
                _, nwin = _pack_bucket(buckets[p][ch])
                mx = max(mx, nwin)
    return mx * W


# --------------------------------------------------------------------------
# device kernel (per core; SPMD across 8 cores)
# --------------------------------------------------------------------------

def _build_kernel(tc: tile.TileContext, out, ins, b_const: float, S: int):
    nc = tc.nc
    vd, xT, wall, dup, attn_p, tags, lsidx = ins
    TS = NB * S
    NWT = TS // W

    with tc.tile_pool(name="small", bufs=1) as sp, \
         tc.tile_pool(name="psum", bufs=1, space="PSUM") as pp, \
         tc.tile_pool(name="stream", bufs=6) as pool, \
         tc.tile_pool(name="sparse", bufs=3) as spp:
        # ---- small input loads (SP queue), p_gen inputs first ----
        wt = sp.tile([P, NK], F32)
        nc.sync.dma_start(wt[:], wall[:, :])
        xt = sp.tile([P, NK * BSH], F32)
        nc.sync.dma_start(xt[:], xT[:, :])
        dupt = sp.tile([BSH, P], F32)
        nc.sync.dma_start(dupt[:], dup[:, :])
        at = sp.tile([P, TS], BF16)
        nc.sync.dma_start(at[:], attn_p[:, :])
        tg = sp.tile([P, TS], BF16)
        nc.sync.dma_start(tg[:], tags[:, :])
        lsi = sp.tile([P, TS], I16)
        nc.sync.dma_start(lsi[:], lsidx[:, :])

        # ---- windowed duplicate-group sums (bf16 eq/mul, f32 reduce) ----
        idw = tg[:].rearrange("p (w i) -> p w i", i=W)
        id_i = idw[:, :, :, None].to_broadcast([P, NWT, W, W])
        id_j = idw[:, :, None, :].to_broadcast([P, NWT, W, W])
        eq = sp.tile([P, NWT * W * W], BF16)
        eqv = eq[:].rearrange("p (w i j) -> p w i j", i=W, j=W)
        nc.vector.tensor_tensor(eqv, id_i, id_j, op=mybir.AluOpType.is_equal)
        at_j = (
            at[:]
            .rearrange("p (w i) -> p w i", i=W)[:, :, None, :]
            .to_broadcast([P, NWT, W, W])
        )
        nc.vector.tensor_mul(eqv, eqv, at_j)
        gs = sp.tile([P, TS], F32)
        nc.vector.reduce_sum(
            gs[:].rearrange("p (w i) -> p w i", i=W),
            eqv,
            axis=mybir.AxisListType.X,
        )

        # ---- p_gen = sigmoid(x @ w + b) via PE (f32) ----
        d64 = pp.tile([BSH, 1], F32, space="PSUM")
        for k in range(NK):
            nc.tensor.matmul(
                d64[:],
                lhsT=xt[:, k * BSH : (k + 1) * BSH],
                rhs=wt[:, k : k + 1],
                start=(k == 0),
                stop=(k == NK - 1),
            )
        d64s = sp.tile([BSH, 1], F32)
        nc.vector.tensor_scalar_mul(d64s[:], d64[:], 1.0)
        dots = pp.tile([P, 1], F32, space="PSUM")
        nc.tensor.matmul(dots[:], lhsT=dupt[:], rhs=d64s[:], start=True, stop=True)
        pgd = sp.tile([P, 1], F32)
        nc.scalar.activation(
            pgd[:], dots[:], mybir.ActivationFunctionType.Sigmoid, bias=b_const
        )
        omd = sp.tile([P, 1], F32)  # 1 - p_gen
        nc.vector.tensor_scalar(
            omd[:], pgd[:], -1.0, 1.0,
            mybir.AluOpType.mult, mybir.AluOpType.add,
        )
        gsc = sp.tile([P, TS], BF16)  # (1 - p_gen) * group sums
        nc.scalar.mul(gsc[:], gs[:], omd[:])

        # ---- stream: out = p_gen * vocab_dist + sparse ----
        vdv = vd.rearrange("(p v) -> p v", p=P)
        outv = out.rearrange("(p v) -> p v", p=P)
        for c in range(NSTREAM):
            tl = pool.tile([P, SW], F32, tag="stream")
            nc.sync.dma_start(tl[:], vdv[:, c * SW : (c + 1) * SW])
            spt = spp.tile([P, SW], BF16, tag="spt")
            nc.gpsimd.local_scatter(
                out_ap=spt[:],
                data_ap=gsc[:, c * S : (c + 1) * S],
                idxs_ap=lsi[:, c * S : (c + 1) * S],
                channels=P, num_elems=SW, num_idxs=S,
            )
            # tl = (tl * p_gen) + sparse, fused on DVE
            nc.vector.scalar_tensor_tensor(
                tl[:], tl[:], pgd[:], spt[:],
                op0=mybir.AluOpType.mult, op1=mybir.AluOpType.add,
            )
            nc.scalar.dma_start(outv[:, c * SW : (c + 1) * SW], tl[:])


# --------------------------------------------------------------------------
# entry point
# --------------------------------------------------------------------------

last_results = None  # BassKernelResults of the most recent run (for benchmarks)


def build_program(b_const: float, S: int):
    nc = bacc.Bacc("TRN2", target_bir_lowering=False, debug=False,
                   num_devices=NCORES)
    vd_t = nc.dram_tensor("vd", [BSH * V], F32, kind="ExternalInput")
    xT_t = nc.dram_tensor("xT", [P, NK * BSH], F32, kind="ExternalInput")
    wall_t = nc.dram_tensor("wall", [P, NK], F32, kind="ExternalInput")
    dup_t = nc.dram_tensor("dup", [BSH, P], F32, kind="ExternalInput")
    attn_t = nc.dram_tensor("attn_p", [P, NB * S], BF16, kind="ExternalInput")
    tags_t = nc.dram_tensor("tags", [P, NB * S], BF16, kind="ExternalInput")
    lsi_t = nc.dram_tensor("lsidx", [P, NB * S], I16, kind="ExternalInput")
    out_t = nc.dram_tensor("out", [BSH * V], F32, kind="ExternalOutput")

    with tile.TileContext(nc) as tc:
        _build_kernel(
            tc,
            out_t.ap(),
            (vd_t.ap(), xT_t.ap(), wall_t.ap(), dup_t.ap(), attn_t.ap(),
             tags_t.ap(), lsi_t.ap()),
            b_const,
            S,
        )
    nc.compile()
    return nc


def prepare_in_maps(vocab_dist, attn_dist, xcat_full, wall_np, src_ids, vs, S):
    # wall laid out [P, NK]: wall[p, k] = w[k*128 + p]
    wall_t = np.ascontiguousarray(wall_np.reshape(NK, P).T)
    # duplication matrix: row b feeds partitions 2b and 2b+1
    dup = np.zeros((BSH, P), np.float32)
    dup[np.arange(BSH), 2 * np.arange(BSH)] = 1.0
    dup[np.arange(BSH), 2 * np.arange(BSH) + 1] = 1.0
    in_maps = []
    for c in range(NCORES):
        sl = slice(c * BSH, (c + 1) * BSH)
        attn_p, tags, lsidx = _prep_shard(attn_dist[sl], src_ids[sl], vs, S)
        # xT laid out [P, NK*BSH]: xT[p, k*BSH + m] = x[m, k*128 + p]
        xT = np.ascontiguousarray(
            xcat_full[sl].T.reshape(NK, P, BSH).transpose(1, 0, 2).reshape(P, -1)
        )
        in_maps.append(
            {
                "vd": np.ascontiguousarray(vocab_dist[sl]).reshape(-1),
                "xT": xT,
                "wall": wall_t,
                "dup": dup,
                "attn_p": attn_p,
                "tags": tags,
                "lsidx": lsidx,
            }
        )
    return in_maps


def kernel(vocab_dist, attn_dist, context, state, emb, src_ids, vocab_size,
           w_c, w_s, w_y, b, **kwargs):
    vocab_dist = np.ascontiguousarray(np.asarray(vocab_dist, dtype=np.float32))
    attn_dist = np.asarray(attn_dist, dtype=np.float32)
    xcat_full = np.ascontiguousarray(
        np.concatenate(
            [np.asarray(context), np.asarray(state), np.asarray(emb)], axis=1
        ).astype(np.float32)
    )
    src_ids = np.asarray(src_ids)
    vs = int(np.asarray(vocab_size))
    wall_np = np.ascontiguousarray(
        np.concatenate(
            [np.asarray(w_c), np.asarray(w_s), np.asarray(w_y)]
        ).astype(np.float32)
    )
    b_const = float(np.asarray(b).reshape(-1)[0])

    assert vocab_dist.shape == (B, V) and attn_dist.shape == (B, T)
    assert xcat_full.shape == (B, D) and src_ids.shape == (B, T)

    S = _slot_requirement(src_ids, vs)
    nc = build_program(b_const, S)
    in_maps = prepare_in_maps(
        vocab_dist, attn_dist, xcat_full, wall_np, src_ids, vs, S
    )

    _trace = os.environ.get("PG_KERNEL_TRACE", "0") == "1"
    res = bass_utils.run_bass_kernel_spmd(
        nc, in_maps, core_ids=list(range(NCORES)), trace=_trace
    )
    global last_results
    last_results = res

    out = np.empty((B, V), np.float32)
    for c in range(NCORES):
        out[c * BSH : (c + 1) * BSH] = res.results[c]["out"].reshape(BSH, V)
    return out


# revision 7
# speedup vs baseline: 1.6508x; 1.2328x over previous
"""Pointer-generator head on 8 Trainium2 NeuronCores (Bass/Tile).

Computation (per batch row b):
    p_gen = sigmoid(context @ w_c + state @ w_s + emb @ w_y + b)
    out   = p_gen * vocab_dist
    out[b, src_ids[b, t]] += (1 - p_gen) * attn_dist[b, t]   (masked, clamped)

Sharding: batch dim (512) split across 8 cores, 64 rows each; every core keeps
its rows' full V=32000 columns so the scatter-add stays core-local; the small
parameter vectors are replicated.

Per-core device kernel (interleaved layout: partition p = 2b+h holds row b's
half-row h = [h*16000, (h+1)*16000) contiguously):

  * p_gen: 20 accumulating fp16 PE matmuls produce per-row dots [64,1] in f32
    PSUM; a 0/1 duplication matmul expands them to the interleaved [128,1]
    layout; sigmoid on the scalar engine.
  * scatter prep: the host buckets the T=400 items per row by 2000-column
    target bucket (8 per partition), laying each duplicate-id group out as
    consecutive slots.  The device computes group sums with a single DVE
    prefix scan (state = cont*state + attn, f32 state): each group's last
    slot holds its total.  Group sums are scaled by (1 - p_gen), in bf16.
  * stream: 8 chunks of [128, 2000] f32.  Per chunk, one GPSIMD local_scatter
    writes the chunk's bf16 group sums into a bf16 sparse tile (slots that
    are not a group's last member carry index -1 and are dropped); one DVE
    scalar_tensor_tensor computes p_gen * dense + sparse, emitting bf16; the
    result streams back out as bf16 and the host widens it to f32 (pure
    dtype relayout).  Dense input DMAs own the SP queue; the packed sideband
    load and output DMAs share the scalar-engine queue.

All sideband metadata (fp16 x^T, fp16 weights, bf16 attn slots, bf16
continuation flags, int16 scatter indices) is bit-packed by the host into a
single [128, *] int16 tensor so one DMA config covers it.  Host-side work is
limited to index metadata and pure data relayout (sharding, transposition,
dtype casts).
"""

import os

import ml_dtypes
import numpy as np

import concourse.bacc as bacc
import concourse.mybir as mybir
import concourse.tile as tile
from concourse import bass_utils

# ---- problem shape (hardcoded per spec) ----
B = 512
T = 400
V = 32000
ENC, HID, EMB = 1024, 1024, 512
NCORES = 8

P = 128
BSH = B // NCORES       # 64 rows per core
HV = V // 2             # half-row width per partition
D = ENC + HID + EMB     # 2560
NK = D // P             # K-chunks for the p_gen matmul
NB = 8                  # scatter buckets per partition
BW = HV // NB           # 2000 f32 columns per bucket
NSTREAM = 8
SW = HV // NSTREAM      # 2000 f32 per partition per stream chunk (== BW)
XW = NK * BSH           # 1280 fp16 x^T columns per partition

F32 = mybir.dt.float32
F16 = mybir.dt.float16
BF16 = mybir.dt.bfloat16
I16 = mybir.dt.int16

NPBF16 = ml_dtypes.bfloat16


# --------------------------------------------------------------------------
# host-side index prep (pure metadata / relayout)
# --------------------------------------------------------------------------

def _bucketize(src_ids: np.ndarray, vocab_size: int):
    """Bucket one shard's unmasked items by (partition, bucket).

    Returns buckets[p][c] = list of (bucket-local target, [t indices]) groups;
    each group shares one raw id.  Masked items (id >= min(vocab_size, V))
    contribute nothing and are dropped.
    """
    id_lim = min(int(vocab_size), V)
    buckets = [[[] for _ in range(NB)] for _ in range(P)]
    for b in range(BSH):
        order: dict[int, list[int]] = {}
        for t, i in enumerate(src_ids[b].tolist()):
            if i < id_lim:
                order.setdefault(i, []).append(t)
        for i, g in order.items():
            h, off = divmod(i, HV)
            c = off // BW
            buckets[2 * b + h][c].append((off - c * BW, g))
    return buckets


def _prep_shard(attn: np.ndarray, src_ids: np.ndarray, vocab_size: int, S: int):
    """-> (attn_p, cont, lsidx): [P, NB*S] bf16, [P, NB*S] bf16, [P, NB*S] i16.

    Slot layout per partition: NB segments of S slots; every duplicate-id
    group occupies consecutive slots.  cont is 0 on each group's first slot
    and 1 on the rest, so the device prefix scan state = cont*state + attn
    leaves the group total on its LAST slot; lsidx carries the group's
    bucket-local f32 target column on that last slot and -1 (dropped)
    everywhere else.  Pad slots have attn 0, cont 0, lsidx -1."""
    TS = NB * S
    attn_f = np.zeros((P, TS), np.float32)
    cont_f = np.zeros((P, TS), np.float32)
    lsidx = np.full((P, TS), -1, np.int16)
    buckets = _bucketize(src_ids, vocab_size)
    for p in range(P):
        row = p // 2
        for c in range(NB):
            j = c * S
            for loc, ts in buckets[p][c]:
                for k, t in enumerate(ts):
                    attn_f[p, j + k] = attn[row, t]
                    cont_f[p, j + k] = 0.0 if k == 0 else 1.0
                j += len(ts)
                lsidx[p, j - 1] = loc
            assert j <= (c + 1) * S
    return attn_f.astype(NPBF16), cont_f.astype(NPBF16), lsidx


def _slot_requirement(src_ids_full: np.ndarray, vocab_size: int):
    """Global S: max items in any (core, partition, bucket), rounded even."""
    mx = 2
    for c in range(NCORES):
        buckets = _bucketize(src_ids_full[c * BSH : (c + 1) * BSH], vocab_size)
        for p in range(P):
            for ch in range(NB):
                n = sum(len(ts) for _, ts in buckets[p][ch])
                mx = max(mx, n)
    return (mx + 1) // 2 * 2


# --------------------------------------------------------------------------
# device kernel (per core; SPMD across 8 cores)
# --------------------------------------------------------------------------

def _build_kernel(tc: tile.TileContext, out, ins, b_const: float, S: int):
    nc = tc.nc
    vd, side, dup = ins
    TS = NB * S
    # packed sideband column offsets (int16 units)
    XT0, W0, AT0, CT0, LS0 = 0, XW, XW + NK, XW + NK + TS, XW + NK + 2 * TS
    NSB = XW + NK + 3 * TS

    with tc.tile_pool(name="small", bufs=1) as sp, \
         tc.tile_pool(name="psum", bufs=1, space="PSUM") as pp, \
         tc.tile_pool(name="stream", bufs=8) as pool, \
         tc.tile_pool(name="sparse", bufs=4) as spp:
        # ---- dense input stream: 8 chunk loads own the SP queue ----
        vdv = vd.rearrange("(p v) -> p v", p=P)
        outv = out.rearrange("(p v) -> p v", p=P)
        tls = []
        for c in range(NSTREAM):
            tl = pool.tile([P, SW], F32, tag="stream")
            nc.sync.dma_start(tl[:], vdv[:, c * SW : (c + 1) * SW])
            tls.append(tl)

        # ---- sideband: one packed load + the tiny dup matrix ----
        sb = sp.tile([P, NSB], I16)
        nc.scalar.dma_start(sb[:], side[:, :])
        dupt = sp.tile([BSH, P], F16)
        nc.scalar.dma_start(dupt[:], dup[:, :])
        xt = sb[:, XT0 : XT0 + XW].bitcast(F16)
        wt = sb[:, W0 : W0 + NK].bitcast(F16)
        at = sb[:, AT0 : AT0 + TS].bitcast(BF16)
        ct = sb[:, CT0 : CT0 + TS].bitcast(BF16)
        lsi = sb[:, LS0 : LS0 + TS]

        # ---- duplicate-group sums via prefix scan (f32 state) ----
        gs = sp.tile([P, TS], F32)
        nc.vector.tensor_tensor_scan(
            gs[:], ct, at, 0.0,
            op0=mybir.AluOpType.mult, op1=mybir.AluOpType.add,
        )

        # ---- p_gen = sigmoid(x @ w + b) via PE (fp16 in, f32 accum) ----
        d64 = pp.tile([BSH, 1], F32, space="PSUM")
        for k in range(NK):
            nc.tensor.matmul(
                d64[:],
                lhsT=xt[:, k * BSH : (k + 1) * BSH],
                rhs=wt[:, k : k + 1],
                start=(k == 0),
                stop=(k == NK - 1),
            )
        d64s = sp.tile([BSH, 1], F16)
        nc.vector.tensor_scalar_mul(d64s[:], d64[:], 1.0)
        dots = pp.tile([P, 1], F32, space="PSUM")
        nc.tensor.matmul(dots[:], lhsT=dupt[:], rhs=d64s[:], start=True, stop=True)
        pgd = sp.tile([P, 1], F32)
        nc.scalar.activation(
            pgd[:], dots[:], mybir.ActivationFunctionType.Sigmoid, bias=b_const
        )
        omd = sp.tile([P, 1], F32)  # 1 - p_gen
        nc.vector.tensor_scalar(
            omd[:], pgd[:], -1.0, 1.0,
            mybir.AluOpType.mult, mybir.AluOpType.add,
        )
        gsc = sp.tile([P, TS], BF16)  # (1 - p_gen) * group sums
        nc.scalar.mul(gsc[:], gs[:], omd[:])

        # ---- stream: out = p_gen * vocab_dist + sparse (bf16 out) ----
        for c in range(NSTREAM):
            tl = tls[c]
            spt = spp.tile([P, SW], BF16, tag="spt")
            nc.gpsimd.local_scatter(
                out_ap=spt[:],
                data_ap=gsc[:, c * S : (c + 1) * S],
                idxs_ap=lsi[:, c * S : (c + 1) * S],
                channels=P, num_elems=SW, num_idxs=S,
            )
            # tlb = (tl * p_gen) + sparse, fused on DVE, bf16 out
            tlb = spp.tile([P, SW], BF16, tag="tlb")
            nc.vector.scalar_tensor_tensor(
                tlb[:], tl[:], pgd[:], spt[:],
                op0=mybir.AluOpType.mult, op1=mybir.AluOpType.add,
            )
            nc.scalar.dma_start(outv[:, c * SW : (c + 1) * SW], tlb[:])


# --------------------------------------------------------------------------
# entry point
# --------------------------------------------------------------------------

last_results = None  # BassKernelResults of the most recent run (for benchmarks)


def build_program(b_const: float, S: int):
    TS = NB * S
    NSB = XW + NK + 3 * TS
    nc = bacc.Bacc("TRN2", target_bir_lowering=False, debug=False,
                   num_devices=NCORES)
    vd_t = nc.dram_tensor("vd", [BSH * V], F32, kind="ExternalInput")
    side_t = nc.dram_tensor("side", [P, NSB], I16, kind="ExternalInput")
    dup_t = nc.dram_tensor("dup", [BSH, P], F16, kind="ExternalInput")
    out_t = nc.dram_tensor("out", [BSH * V], BF16, kind="ExternalOutput")

    with tile.TileContext(nc) as tc:
        _build_kernel(
            tc,
            out_t.ap(),
            (vd_t.ap(), side_t.ap(), dup_t.ap()),
            b_const,
            S,
        )
    nc.compile()
    return nc


def prepare_in_maps(vocab_dist, attn_dist, xcat_full, wall_np, src_ids, vs, S):
    # wall laid out [P, NK]: wall[p, k] = w[k*128 + p]
    wall_t = np.ascontiguousarray(wall_np.reshape(NK, P).T).astype(np.float16)
    # duplication matrix: row b feeds partitions 2b and 2b+1
    dup = np.zeros((BSH, P), np.float16)
    dup[np.arange(BSH), 2 * np.arange(BSH)] = 1.0
    dup[np.arange(BSH), 2 * np.arange(BSH) + 1] = 1.0
    in_maps = []
    for c in range(NCORES):
        sl = slice(c * BSH, (c + 1) * BSH)
        attn_p, cont, lsidx = _prep_shard(attn_dist[sl], src_ids[sl], vs, S)
        # xT laid out [P, NK*BSH]: xT[p, k*BSH + m] = x[m, k*128 + p]
        xT = np.ascontiguousarray(
            xcat_full[sl].T.reshape(NK, P, BSH).transpose(1, 0, 2).reshape(P, -1)
        ).astype(np.float16)
        side = np.concatenate(
            [
                xT.view(np.int16),
                wall_t.view(np.int16),
                attn_p.view(np.int16),
                cont.view(np.int16),
                lsidx,
            ],
            axis=1,
        )
        in_maps.append(
            {
                "vd": np.ascontiguousarray(vocab_dist[sl]).reshape(-1),
                "side": np.ascontiguousarray(side),
                "dup": dup,
            }
        )
    return in_maps


def kernel(vocab_dist, attn_dist, context, state, emb, src_ids, vocab_size,
           w_c, w_s, w_y, b, **kwargs):
    vocab_dist = np.ascontiguousarray(np.asarray(vocab_dist, dtype=np.float32))
    attn_dist = np.asarray(attn_dist, dtype=np.float32)
    xcat_full = np.ascontiguousarray(
        np.concatenate(
            [np.asarray(context), np.asarray(state), np.asarray(emb)], axis=1
        ).astype(np.float32)
    )
    src_ids = np.asarray(src_ids)
    vs = int(np.asarray(vocab_size))
    wall_np = np.ascontiguousarray(
        np.concatenate(
            [np.asarray(w_c), np.asarray(w_s), np.asarray(w_y)]
        ).astype(np.float32)
    )
    b_const = float(np.asarray(b).reshape(-1)[0])

    assert vocab_dist.shape == (B, V) and attn_dist.shape == (B, T)
    assert xcat_full.shape == (B, D) and src_ids.shape == (B, T)

    S = _slot_requirement(src_ids, vs)
    nc = build_program(b_const, S)
    in_maps = prepare_in_maps(
        vocab_dist, attn_dist, xcat_full, wall_np, src_ids, vs, S
    )

    _trace = os.environ.get("PG_KERNEL_TRACE", "0") == "1"
    res = bass_utils.run_bass_kernel_spmd(
        nc, in_maps, core_ids=list(range(NCORES)), trace=_trace
    )
    global last_results
    last_results = res

    out = np.empty((B, V), np.float32)
    for c in range(NCORES):
        out[c * BSH : (c + 1) * BSH] = (
            res.results[c]["out"].astype(np.float32).reshape(BSH, V)
        )
    return out


# revision 8
# speedup vs baseline: 1.7900x; 1.0843x over previous
"""Pointer-generator head on 8 Trainium2 NeuronCores (Bass/Tile).

Computation (per batch row b):
    p_gen = sigmoid(context @ w_c + state @ w_s + emb @ w_y + b)
    out   = p_gen * vocab_dist
    out[b, src_ids[b, t]] += (1 - p_gen) * attn_dist[b, t]   (masked, clamped)

Sharding: batch dim (512) split across 8 cores, 64 rows each; every core keeps
its rows' full V=32000 columns so the scatter-add stays core-local; the small
parameter vectors are replicated.

Per-core device kernel (interleaved layout: partition p = 2b+h holds row b's
half-row h = [h*16000, (h+1)*16000) contiguously):

  * p_gen: 20 accumulating fp16 PE matmuls produce per-row dots [64,1] in f32
    PSUM; a 0/1 duplication matmul expands them to the interleaved [128,1]
    layout; sigmoid on the scalar engine.
  * scatter prep: the host buckets the T=400 items per row by 2000-column
    target bucket (8 per partition), laying each duplicate-id group out as
    consecutive slots.  The device computes group sums with a single DVE
    prefix scan (state = cont*state + attn, f32 state): each group's last
    slot holds its total.  Group sums are scaled by (1 - p_gen), in bf16.
  * stream: 8 chunks of [128, 2000] f32.  Per chunk, one GPSIMD local_scatter
    writes the chunk's bf16 group sums into a bf16 sparse tile (slots that
    are not a group's last member carry index -1 and are dropped); one DVE
    scalar_tensor_tensor computes p_gen * dense + sparse, emitting bf16; the
    result streams back out as bf16 and the host widens it to f32 (pure
    dtype relayout).  The packed sideband load and the dense input stream own
    the SP queue (sideband first); output DMAs use the scalar-engine queue.

All sideband metadata (fp16 x^T, fp16 weights, bf16 attn slots, bf16
continuation flags, int16 scatter indices) is bit-packed by the host into a
single [128, *] int16 tensor so one DMA config covers it.  Host-side work is
limited to index metadata and pure data relayout (sharding, transposition,
dtype casts).
"""

import os

import ml_dtypes
import numpy as np

import concourse.bacc as bacc
import concourse.mybir as mybir
import concourse.tile as tile
from concourse import bass_utils

# ---- problem shape (hardcoded per spec) ----
B = 512
T = 400
V = 32000
ENC, HID, EMB = 1024, 1024, 512
NCORES = 8

P = 128
BSH = B // NCORES       # 64 rows per core
HV = V // 2             # half-row width per partition
D = ENC + HID + EMB     # 2560
NK = D // P             # K-chunks for the p_gen matmul
NB = 8                  # scatter buckets per partition
BW = HV // NB           # 2000 f32 columns per bucket
NSTREAM = 8
SW = HV // NSTREAM      # 2000 f32 per partition per stream chunk (== BW)
XW = NK * BSH           # 1280 fp16 x^T columns per partition

F32 = mybir.dt.float32
F16 = mybir.dt.float16
BF16 = mybir.dt.bfloat16
I16 = mybir.dt.int16

NPBF16 = ml_dtypes.bfloat16


# --------------------------------------------------------------------------
# host-side index prep (pure metadata / relayout)
# --------------------------------------------------------------------------

def _bucketize(src_ids: np.ndarray, vocab_size: int):
    """Bucket one shard's unmasked items by (partition, bucket).

    Returns buckets[p][c] = list of (bucket-local target, [t indices]) groups;
    each group shares one raw id.  Masked items (id >= min(vocab_size, V))
    contribute nothing and are dropped.
    """
    id_lim = min(int(vocab_size), V)
    buckets = [[[] for _ in range(NB)] for _ in range(P)]
    for b in range(BSH):
        order: dict[int, list[int]] = {}
        for t, i in enumerate(src_ids[b].tolist()):
            if i < id_lim:
                order.setdefault(i, []).append(t)
        for i, g in order.items():
            h, off = divmod(i, HV)
            c = off // BW
            buckets[2 * b + h][c].append((off - c * BW, g))
    return buckets


def _prep_shard(attn: np.ndarray, src_ids: np.ndarray, vocab_size: int, S: int):
    """-> (attn_p, cont, lsidx): [P, NB*S] bf16, [P, NB*S] bf16, [P, NB*S] i16.

    Slot layout per partition: NB segments of S slots; every duplicate-id
    group occupies consecutive slots.  cont is 0 on each group's first slot
    and 1 on the rest, so the device prefix scan state = cont*state + attn
    leaves the group total on its LAST slot; lsidx carries the group's
    bucket-local f32 target column on that last slot and -1 (dropped)
    everywhere else.  Pad slots have attn 0, cont 0, lsidx -1."""
    TS = NB * S
    attn_f = np.zeros((P, TS), np.float32)
    cont_f = np.zeros((P, TS), np.float32)
    lsidx = np.full((P, TS), -1, np.int16)
    buckets = _bucketize(src_ids, vocab_size)
    for p in range(P):
        row = p // 2
        for c in range(NB):
            j = c * S
            for loc, ts in buckets[p][c]:
                for k, t in enumerate(ts):
                    attn_f[p, j + k] = attn[row, t]
                    cont_f[p, j + k] = 0.0 if k == 0 else 1.0
                j += len(ts)
                lsidx[p, j - 1] = loc
            assert j <= (c + 1) * S
    return attn_f.astype(NPBF16), cont_f.astype(NPBF16), lsidx


def _slot_requirement(src_ids_full: np.ndarray, vocab_size: int):
    """Global S: max items in any (core, partition, bucket), rounded even."""
    mx = 2
    for c in range(NCORES):
        buckets = _bucketize(src_ids_full[c * BSH : (c + 1) * BSH], vocab_size)
        for p in range(P):
            for ch in range(NB):
                n = sum(len(ts) for _, ts in buckets[p][ch])
                mx = max(mx, n)
    return (mx + 1) // 2 * 2


# --------------------------------------------------------------------------
# device kernel (per core; SPMD across 8 cores)
# --------------------------------------------------------------------------

def _build_kernel(tc: tile.TileContext, out, ins, b_const: float, S: int):
    nc = tc.nc
    vd, side, dup = ins
    TS = NB * S
    # packed sideband column offsets (int16 units)
    XT0, W0, AT0, CT0, LS0 = 0, XW, XW + NK, XW + NK + TS, XW + NK + 2 * TS
    NSB = XW + NK + 3 * TS

    with tc.tile_pool(name="small", bufs=1) as sp, \
         tc.tile_pool(name="psum", bufs=1, space="PSUM") as pp, \
         tc.tile_pool(name="stream", bufs=8) as pool, \
         tc.tile_pool(name="sparse", bufs=4) as spp:
        # ---- sideband first on the SP queue: one packed load + dup ----
        sb = sp.tile([P, NSB], I16)
        nc.sync.dma_start(sb[:], side[:, :])
        dupt = sp.tile([BSH, P], F16)
        nc.sync.dma_start(dupt[:], dup[:, :])

        # ---- dense input stream: 8 chunk loads follow on the SP queue ----
        vdv = vd.rearrange("(p v) -> p v", p=P)
        outv = out.rearrange("(p v) -> p v", p=P)
        tls = []
        for c in range(NSTREAM):
            tl = pool.tile([P, SW], F32, tag="stream")
            nc.sync.dma_start(tl[:], vdv[:, c * SW : (c + 1) * SW])
            tls.append(tl)
        xt = sb[:, XT0 : XT0 + XW].bitcast(F16)
        wt = sb[:, W0 : W0 + NK].bitcast(F16)
        at = sb[:, AT0 : AT0 + TS].bitcast(BF16)
        ct = sb[:, CT0 : CT0 + TS].bitcast(BF16)
        lsi = sb[:, LS0 : LS0 + TS]

        # ---- duplicate-group sums via prefix scan (f32 state) ----
        gs = sp.tile([P, TS], F32)
        nc.vector.tensor_tensor_scan(
            gs[:], ct, at, 0.0,
            op0=mybir.AluOpType.mult, op1=mybir.AluOpType.add,
        )

        # ---- p_gen = sigmoid(x @ w + b) via PE (fp16 in, f32 accum) ----
        d64 = pp.tile([BSH, 1], F32, space="PSUM")
        for k in range(NK):
            nc.tensor.matmul(
                d64[:],
                lhsT=xt[:, k * BSH : (k + 1) * BSH],
                rhs=wt[:, k : k + 1],
                start=(k == 0),
                stop=(k == NK - 1),
            )
        d64s = sp.tile([BSH, 1], F16)
        nc.vector.tensor_scalar_mul(d64s[:], d64[:], 1.0)
        dots = pp.tile([P, 1], F32, space="PSUM")
        nc.tensor.matmul(dots[:], lhsT=dupt[:], rhs=d64s[:], start=True, stop=True)
        pgd = sp.tile([P, 1], F32)
        nc.scalar.activation(
            pgd[:], dots[:], mybir.ActivationFunctionType.Sigmoid, bias=b_const
        )
        omd = sp.tile([P, 1], F32)  # 1 - p_gen
        nc.vector.tensor_scalar(
            omd[:], pgd[:], -1.0, 1.0,
            mybir.AluOpType.mult, mybir.AluOpType.add,
        )
        gsc = sp.tile([P, TS], BF16)  # (1 - p_gen) * group sums
        nc.scalar.mul(gsc[:], gs[:], omd[:])

        # ---- stream: out = p_gen * vocab_dist + sparse (bf16 out) ----
        for c in range(NSTREAM):
            tl = tls[c]
            spt = spp.tile([P, SW], BF16, tag="spt")
            nc.gpsimd.local_scatter(
                out_ap=spt[:],
                data_ap=gsc[:, c * S : (c + 1) * S],
                idxs_ap=lsi[:, c * S : (c + 1) * S],
                channels=P, num_elems=SW, num_idxs=S,
            )
            # tlb = (tl * p_gen) + sparse, fused on DVE, bf16 out
            tlb = spp.tile([P, SW], BF16, tag="tlb")
            nc.vector.scalar_tensor_tensor(
                tlb[:], tl[:], pgd[:], spt[:],
                op0=mybir.AluOpType.mult, op1=mybir.AluOpType.add,
            )
            nc.scalar.dma_start(outv[:, c * SW : (c + 1) * SW], tlb[:])


# --------------------------------------------------------------------------
# entry point
# --------------------------------------------------------------------------

last_results = None  # BassKernelResults of the most recent run (for benchmarks)


def build_program(b_const: float, S: int):
    TS = NB * S
    NSB = XW + NK + 3 * TS
    nc = bacc.Bacc("TRN2", target_bir_lowering=False, debug=False,
                   num_devices=NCORES)
    vd_t = nc.dram_tensor("vd", [BSH * V], F32, kind="ExternalInput")
    side_t = nc.dram_tensor("side", [P, NSB], I16, kind="ExternalInput")
    dup_t = nc.dram_tensor("dup", [BSH, P], F16, kind="ExternalInput")
    out_t = nc.dram_tensor("out", [BSH * V], BF16, kind="ExternalOutput")

    with tile.TileContext(nc) as tc:
        _build_kernel(
            tc,
            out_t.ap(),
            (vd_t.ap(), side_t.ap(), dup_t.ap()),
            b_const,
            S,
        )
    nc.compile()
    return nc


def prepare_in_maps(vocab_dist, attn_dist, xcat_full, wall_np, src_ids, vs, S):
    # wall laid out [P, NK]: wall[p, k] = w[k*128 + p]
    wall_t = np.ascontiguousarray(wall_np.reshape(NK, P).T).astype(np.float16)
    # duplication matrix: row b feeds partitions 2b and 2b+1
    dup = np.zeros((BSH, P), np.float16)
    dup[np.arange(BSH), 2 * np.arange(BSH)] = 1.0
    dup[np.arange(BSH), 2 * np.arange(BSH) + 1] = 1.0
    in_maps = []
    for c in range(NCORES):
        sl = slice(c * BSH, (c + 1) * BSH)
        attn_p, cont, lsidx = _prep_shard(attn_dist[sl], src_ids[sl], vs, S)
        # xT laid out [P, NK*BSH]: xT[p, k*BSH + m] = x[m, k*128 + p]
        xT = np.ascontiguousarray(
            xcat_full[sl].T.reshape(NK, P, BSH).transpose(1, 0, 2).reshape(P, -1)
        ).astype(np.float16)
        side = np.concatenate(
            [
                xT.view(np.int16),
                wall_t.view(np.int16),
                attn_p.view(np.int16),
                cont.view(np.int16),
                lsidx,
            ],
            axis=1,
        )
        in_maps.append(
            {
                "vd": np.ascontiguousarray(vocab_dist[sl]).reshape(-1),
                "side": np.ascontiguousarray(side),
                "dup": dup,
            }
        )
    return in_maps


def kernel(vocab_dist, attn_dist, context, state, emb, src_ids, vocab_size,
           w_c, w_s, w_y, b, **kwargs):
    vocab_dist = np.ascontiguousarray(np.asarray(vocab_dist, dtype=np.float32))
    attn_dist = np.asarray(attn_dist, dtype=np.float32)
    xcat_full = np.ascontiguousarray(
        np.concatenate(
            [np.asarray(context), np.asarray(state), np.asarray(emb)], axis=1
        ).astype(np.float32)
    )
    src_ids = np.asarray(src_ids)
    vs = int(np.asarray(vocab_size))
    wall_np = np.ascontiguousarray(
        np.concatenate(
            [np.asarray(w_c), np.asarray(w_s), np.asarray(w_y)]
        ).astype(np.float32)
    )
    b_const = float(np.asarray(b).reshape(-1)[0])

    assert vocab_dist.shape == (B, V) and attn_dist.shape == (B, T)
    assert xcat_full.shape == (B, D) and src_ids.shape == (B, T)

    S = _slot_requirement(src_ids, vs)
    nc = build_program(b_const, S)
    in_maps = prepare_in_maps(
        vocab_dist, attn_dist, xcat_full, wall_np, src_ids, vs, S
    )

    _trace = os.environ.get("PG_KERNEL_TRACE", "0") == "1"
    res = bass_utils.run_bass_kernel_spmd(
        nc, in_maps, core_ids=list(range(NCORES)), trace=_trace
    )
    global last_results
    last_results = res

    out = np.empty((B, V), np.float32)
    for c in range(NCORES):
        out[c * BSH : (c + 1) * BSH] = (
            res.results[c]["out"].astype(np.float32).reshape(BSH, V)
        )
    return out
